# revision 1
# baseline (speedup 1.0000x reference)
"""DifferentialDropout Trainium2 kernel (8-core SPMD).

Reference semantics (see problem): per-row corrcoef factor, global-standardized
1000-bin per-row histograms -> entropies -> per-row keep prob -> mask+scale.

Sharding:
  Phase A (D-shard): each core takes a 2048-col slice of temp [1024, 16384]:
    partial row sums / global sum/sumsq/min/max (AllGather + local reduce),
    partial cov = xs @ xs.T via PE (K-sharded GEMM) -> AllReduce (overlapped
    with phase B histogram). Centering applied post-AR as rank-1 correction:
    cov = xxT - outer(rowsum, rowsum)/D.
  Phase B (B-shard): each core owns 128 rows: exact per-row 1000-bin histogram
    (radix 32x32: bf16 one-hot planes on DVE, combined per 128-element k-tile
    by PE matmuls accumulating [32q, 32l] counts in PSUM), entropies, factors,
    and the final mask/scale pass.
"""

import sys

sys.path.insert(0, "/opt/trn_rl_repo")

import numpy as np
import os

B = 1024
D = 16384
BINS = 1000
N_CORES = 8
DSL = D // N_CORES      # 2048
RSL = B // N_CORES      # 128
import os
C_ROWS = int(os.environ.get("K_CROWS", "4"))   # rows per histogram chunk
ABL = os.environ.get("K_ABL", "")              # ablation: skip phases in sim
DEBUG = os.environ.get("K_DEBUG", "0") == "1"
N_GPS = int(os.environ.get("K_NGPS", "14"))    # L-planes on gpsimd
N_ACT = int(os.environ.get("K_NACT", "8"))     # L-planes on scalar engine
EW_BUFS = int(os.environ.get("K_EWB", "10"))
F = C_ROWS * 128        # free elems/partition/chunk
N_CH = RSL // C_ROWS
N_CH_RUN = int(os.environ.get("K_NCH", "0")) or None
LN2 = 0.6931471805599453

_cache = {}


def _build():
    import concourse.mybir as mybir
    import concourse.tile as tile
    from concourse import bacc
    from concourse.masks import make_identity

    F32 = mybir.dt.float32
    BF16 = mybir.dt.bfloat16
    A = mybir.AluOpType
    AF = mybir.ActivationFunctionType
    AX = mybir.AxisListType.X

    nc = bacc.Bacc("TRN2", target_bir_lowering=False, debug=False,
                   num_devices=N_CORES)

    xst = nc.dram_tensor("xst", [DSL, B], F32, kind="ExternalInput")
    xr = nc.dram_tensor("xr", [RSL, D], F32, kind="ExternalInput")
    ur = nc.dram_tensor("ur", [RSL, D], F32, kind="ExternalInput")
    sel = nc.dram_tensor("sel", [128, 8], F32, kind="ExternalInput")
    out = nc.dram_tensor("out", [RSL, D], F32, kind="ExternalOutput")
    if DEBUG:
        dbg = nc.dram_tensor("dbg", [128, 16], F32, kind="ExternalOutput")
        dbg2 = nc.dram_tensor("dbg2", [128, 48], F32, kind="ExternalOutput")
        dbg3 = nc.dram_tensor("dbg3", [128, 12], F32, kind="ExternalOutput")

    xr_v = xr.ap().rearrange("r (p e) -> p r e", p=128)   # [128, 128, 128]

    with tile.TileContext(nc) as tc:
        with (
            tc.tile_pool(name="const", bufs=1) as constp,
            tc.tile_pool(name="persist", bufs=1) as persist,
            tc.tile_pool(name="dram", bufs=1, space="DRAM") as dram,
        ):
            id128 = constp.tile([128, 128], F32, name="id128")
            make_identity(nc, id128[:])
            ones32 = constp.tile([32, 1], F32, name="ones32")
            nc.vector.memset(ones32[:], 1.0)
            epsb = constp.tile([128, 1], F32, name="epsb")
            nc.vector.memset(epsb[:], 1e-30)
            one1 = constp.tile([128, 1], F32, name="one1")
            nc.vector.memset(one1[:], 1.0)
            negq = constp.tile([128, 32], F32, name="negq")
            for _q in range(32):
                nc.vector.memset(negq[:, _q:_q + 1], -float(_q))

            # persistent SBUF
            counts_sb = persist.tile([32, RSL * 32], F32, name="counts_sb")
            scal = persist.tile([128, 24], F32, name="scal")  # scalar consts
            ag_sb = persist.tile([128, 16], F32, name="ag_sb")
            agg_sb = persist.tile([128, 8, 16], F32, name="agg_sb")
            rowsum = persist.tile([128, 8], F32, name="rowsum")
            rsb = persist.tile([128, 1024], F32, name="rsb")
            rdb = persist.tile([128, 1024], F32, name="rdb")
            dcol = persist.tile([128, 8], F32, name="dcol")
            f1col = persist.tile([128, 8], F32, name="f1col")
            sel_sb = persist.tile([128, 8], F32, name="sel_sb")
            pvec = persist.tile([128, 4], F32, name="pvec")  # p, rkeep, keep, f1own
            bc_part = persist.tile([32, 32], F32, name="bc_part")

            nc.sync.dma_start(sel_sb[:], sel.ap())

            # DRAM bounces
            ag_in = dram.tile([128, 16], F32, name="ag_in")
            ag_out = dram.tile([1024, 16], F32, addr_space="Shared", name="ag_out")
            cov_in = dram.tile([1024, 1024], F32, name="cov_in")
            cov_out = dram.tile([128, 1024], F32, name="cov_out")
            agd_in = dram.tile([128, 33], F32, name="agd_in")
            agd_out = dram.tile([1024, 33], F32, addr_space="Shared", name="agd_out")

            # ---------------- Phase A: stats + GEMM --------------------------
            do_a = "A" not in ABL
            do_b = "B" not in ABL
            do_c = "C" not in ABL
            with (
                tc.tile_pool(name="pa_io", bufs=3) as pa_io,
                tc.tile_pool(name="pa_big", bufs=1) as pa_big,
                tc.tile_pool(name="pa_ps2", bufs=2, space="PSUM") as pa_ps2,
                tc.tile_pool(name="pa_ps", bufs=2, space="PSUM") as pa_ps,
                tc.tile_pool(name="pa_w", bufs=2) as pa_w,
            ):
                # stats over own rows (xr): rowsum, sumsq, min, max -> ag_sb[128,16]
                xst_sb = pa_big.tile([128, 16, 1024], F32, name="xst_sb")
                for k in range(16):
                    nc.sync.dma_start(xst_sb[:, k, :],
                                      xst.ap()[k * 128:(k + 1) * 128, :])
                nc.vector.memset(ag_sb[:, 8:12], 3.4e38)
                nc.vector.memset(ag_sb[:, 12:16], -3.4e38)
                SCH = 4096
                for sc_ in range(4):
                    xrs = pa_io.tile([128, SCH], F32, name="xrs")
                    nc.sync.dma_start(xrs[:], xr.ap()[:, sc_ * SCH:(sc_ + 1) * SCH])
                    rs_scr = pa_w.tile([128, SCH], F32, name="rs_scr", tag="scr", bufs=2)
                    nc.vector.tensor_scalar(rs_scr[:], xrs[:], 1.0, 0.0, A.mult, A.add,
                                            accum_out=ag_sb[:, sc_:sc_ + 1])
                    sq_scr = pa_w.tile([128, SCH], F32, name="sq_scr", tag="scr", bufs=2)
                    nc.scalar.activation(sq_scr[:], xrs[:], AF.Square,
                                         accum_out=ag_sb[:, 4 + sc_:5 + sc_])
                    nc.vector.reduce_sum(ag_sb[:, 8 + sc_:9 + sc_], xrs[:], axis=AX,
                                         op=A.min)
                    nc.vector.reduce_sum(ag_sb[:, 12 + sc_:13 + sc_], xrs[:], axis=AX,
                                         op=A.max)
                nc.sync.dma_start(ag_in[:], ag_sb[:])
                nc.gpsimd.collective_compute(
                    "AllGather", A.bypass,
                    replica_groups=[list(range(N_CORES))],
                    ins=[ag_in.opt()], outs=[ag_out.opt()])
                nc.sync.dma_start(
                    agg_sb[:], ag_out[:].rearrange("(r p) c -> p r c", p=128))

                # GEMM on host-transposed slice
                covp = pa_big.tile([128, 8, 1024], F32, name="covp")
                for m in range(8 if "G" not in ABL else 1):
                    for n2 in range(2):
                        pg = pa_ps2.tile([128, 512], F32, name="pg")
                        for k in range(16):
                            nc.tensor.matmul(
                                pg[:],
                                xst_sb[:, k, m * 128:(m + 1) * 128],
                                xst_sb[:, k, n2 * 512:(n2 + 1) * 512],
                                start=(k == 0), stop=(k == 15))
                        nc.scalar.copy(covp[:, m, n2 * 512:(n2 + 1) * 512], pg[:])
                nc.sync.dma_start(
                    cov_in[:].rearrange("(m p) j -> p m j", p=128), covp[:])
                nc.gpsimd.collective_compute(
                    "ReduceScatter", A.add,
                    replica_groups=[list(range(N_CORES))],
                    ins=[cov_in.opt()], outs=[cov_out.opt()])

                # -------- stats reduction + scalar constants --------
                nc.vector.reduce_sum(
                    rowsum[:], agg_sb[:, :, 0:4], axis=AX)
                nc.vector.reduce_sum(scal[:, 16:17], rowsum[:], axis=AX)
                nc.gpsimd.partition_all_reduce(scal[:, 0:1], scal[:, 16:17], 128,
                                               _reduce_add())
                nc.vector.reduce_sum(
                    scal[:, 17:18], agg_sb[:, :, 4:8], axis=mybir.AxisListType.XY)
                nc.gpsimd.partition_all_reduce(scal[:, 1:2], scal[:, 17:18], 128,
                                               _reduce_add())
                nc.vector.reduce_sum(
                    scal[:, 18:19], agg_sb[:, :, 8:12], axis=mybir.AxisListType.XY,
                    op=A.min)
                nc.vector.tensor_single_scalar(scal[:, 18:19], scal[:, 18:19], -1.0, A.mult)
                nc.gpsimd.partition_all_reduce(scal[:, 2:3], scal[:, 18:19], 128,
                                               _reduce_max())
                nc.vector.reduce_sum(
                    scal[:, 19:20], agg_sb[:, :, 12:16], axis=mybir.AxisListType.XY,
                    op=A.max)
                nc.gpsimd.partition_all_reduce(scal[:, 3:4], scal[:, 19:20], 128,
                                               _reduce_max())

                N_f = float(B) * float(D)
                # mu = gsum/N
                nc.vector.tensor_single_scalar(scal[:, 4:5], scal[:, 0:1], 1.0 / N_f, A.mult)
                # var = (gss - gsum*mu)/(N-1); sd = sqrt
                nc.vector.tensor_mul(scal[:, 20:21], scal[:, 0:1], scal[:, 4:5])
                nc.vector.tensor_sub(scal[:, 20:21], scal[:, 1:2], scal[:, 20:21])
                nc.vector.tensor_single_scalar(scal[:, 20:21], scal[:, 20:21],
                                               1.0 / (N_f - 1.0), A.mult)
                nc.scalar.activation(scal[:, 5:6], scal[:, 20:21], AF.Sqrt)
                nc.vector.reciprocal(scal[:, 6:7], scal[:, 5:6])
                # lo = (tmin - mu)*rsd ; tmin = -negmn
                nc.vector.tensor_single_scalar(scal[:, 21:22], scal[:, 2:3], -1.0, A.mult)
                nc.vector.tensor_sub(scal[:, 21:22], scal[:, 21:22], scal[:, 4:5])
                nc.vector.tensor_mul(scal[:, 7:8], scal[:, 21:22], scal[:, 6:7])
                # hi = (tmax - mu)*rsd ; width = (hi - lo)/BINS
                nc.vector.tensor_sub(scal[:, 22:23], scal[:, 3:4], scal[:, 4:5])
                nc.vector.tensor_mul(scal[:, 22:23], scal[:, 22:23], scal[:, 6:7])
                nc.vector.tensor_sub(scal[:, 22:23], scal[:, 22:23], scal[:, 7:8])
                nc.vector.tensor_single_scalar(scal[:, 8:9], scal[:, 22:23],
                                               1.0 / BINS, A.mult)
                nc.vector.reciprocal(scal[:, 9:10], scal[:, 8:9])
                # SC = rsd*rwidth ; BC = -(mu*rsd + lo)*rwidth
                nc.vector.tensor_mul(scal[:, 10:11], scal[:, 6:7], scal[:, 9:10])
                nc.vector.tensor_mul(scal[:, 23:24], scal[:, 4:5], scal[:, 6:7])
                nc.vector.tensor_add(scal[:, 23:24], scal[:, 23:24], scal[:, 7:8])
                nc.vector.tensor_mul(scal[:, 23:24], scal[:, 23:24], scal[:, 9:10])
                nc.vector.tensor_single_scalar(scal[:, 11:12], scal[:, 23:24], -1.0, A.mult)
                # entropy consts: rnw_l = 1/(width*D), rnw_b = 1/(width*N)
                nc.vector.tensor_single_scalar(scal[:, 16:17], scal[:, 8:9], float(D), A.mult)
                nc.vector.reciprocal(scal[:, 12:13], scal[:, 16:17])
                nc.vector.tensor_single_scalar(scal[:, 17:18], scal[:, 8:9], N_f, A.mult)
                nc.vector.reciprocal(scal[:, 13:14], scal[:, 17:18])
                nc.vector.tensor_single_scalar(scal[:, 14:15], scal[:, 12:13],
                                               -1.0 / LN2, A.mult)
                nc.vector.tensor_single_scalar(scal[:, 15:16], scal[:, 13:14],
                                               -1.0 / LN2, A.mult)
                # rsb = broadcast of flattened rowsum
                _bcast_cols(nc, pa_w, pa_ps, rowsum, rsb, id128)

            # ---------------- Phase B: histogram ----------------------------
            with (
                tc.tile_pool(name="hb_io", bufs=3) as hb_io,
                tc.tile_pool(name="hb_w", bufs=2) as hb_w,
                tc.tile_pool(name="hb_pl", bufs=2) as hb_pl,
                tc.tile_pool(name="hb_ps", bufs=8, space="PSUM") as hb_ps,
            ):
                for ch in range(min(N_CH, N_CH_RUN or N_CH) if do_b else 0):
                    r0 = ch * C_ROWS
                    xch = hb_io.tile([128, C_ROWS, 128], F32, name="xch")
                    nc.sync.dma_start(xch[:], xr_v[:, r0:r0 + C_ROWS, :])
                    xf = xch[:].rearrange("p a b -> p (a b)")

                    v = hb_w.tile([128, F], F32, name="v", tag="ew", bufs=EW_BUFS)
                    nc.scalar.activation(v[:], xf, AF.Identity,
                                         bias=scal[:, 11:12], scale=scal[:, 10:11])
                    i1 = hb_w.tile([128, F], mybir.dt.int32, name="i1", tag="ew", bufs=EW_BUFS)
                    nc.vector.tensor_copy(i1[:], v[:])
                    f1t = hb_w.tile([128, F], F32, name="f1t", tag="ew", bufs=EW_BUFS)
                    nc.vector.tensor_copy(f1t[:], i1[:])
                    g1 = hb_w.tile([128, F], F32, name="g1", tag="ew", bufs=EW_BUFS)
                    nc.vector.tensor_tensor(g1[:], f1t[:], v[:], A.is_gt)
                    idx = hb_w.tile([128, F], F32, name="idx", tag="ew", bufs=EW_BUFS)
                    nc.vector.tensor_sub(idx[:], f1t[:], g1[:])
                    i3 = hb_w.tile([128, F], mybir.dt.int32, name="i3", tag="ew", bufs=EW_BUFS)
                    nc.vector.tensor_scalar(i3[:], idx[:], 999.0, 0.0, A.min, A.max)
                    ih = hb_w.tile([128, F], mybir.dt.int32, name="ih", tag="ew", bufs=EW_BUFS)
                    nc.vector.tensor_single_scalar(ih[:], i3[:], 5, A.arith_shift_right)
                    il = hb_w.tile([128, F], mybir.dt.int32, name="il", tag="ew", bufs=EW_BUFS)
                    nc.vector.tensor_single_scalar(il[:], i3[:], 31, A.bitwise_and)
                    hi_b = hb_w.tile([128, F], BF16, name="hi_b")
                    nc.vector.tensor_copy(hi_b[:], ih[:])
                    lo_b = hb_w.tile([128, F], BF16, name="lo_b")
                    nc.vector.tensor_copy(lo_b[:], il[:])

                    Hpl = hb_pl.tile([128, 32 * F], BF16, name="Hpl")
                    Lpl = hb_pl.tile([128, 32 * F], BF16, name="Lpl")
                    for q in range(32):
                        nc.vector.tensor_single_scalar(
                            Hpl[:, q * F:(q + 1) * F], hi_b[:], float(q), A.is_equal)
                    for q in range(N_GPS):
                        nc.gpsimd.tensor_single_scalar(
                            Lpl[:, q * F:(q + 1) * F], lo_b[:], float(q), A.is_equal)
                    for q in range(N_GPS, N_GPS + N_ACT):
                        atmp = hb_w.tile([128, F], BF16, name="atmp")
                        nc.scalar.activation(atmp[:], lo_b[:], AF.Square,
                                             bias=negq[:, q:q + 1])
                        nc.scalar.activation(Lpl[:, q * F:(q + 1) * F], atmp[:],
                                             AF.Relu, bias=one1[:], scale=-1.0)
                    for q in range(N_GPS + N_ACT, 32):
                        nc.vector.tensor_single_scalar(
                            Lpl[:, q * F:(q + 1) * F], lo_b[:], float(q), A.is_equal)

                    Hv = Hpl[:].rearrange("p (q f) -> p f q", q=32)
                    Lv = Lpl[:].rearrange("p (q f) -> p f q", q=32)
                    for r in range(C_ROWS):
                        ps = hb_ps.tile([32, 32], F32, name="ps")
                        for k in range(128):
                            t = r * 128 + k
                            nc.tensor.matmul(ps[:], Hv[:, t, :], Lv[:, t, :],
                                             start=(k == 0), stop=(k == 127))
                        rr = r0 + r
                        nc.scalar.copy(counts_sb[:, rr * 32:(rr + 1) * 32], ps[:32, :])

                # batch-count partial (joins the merged AllGather in phase C)
                nc.vector.reduce_sum(
                    bc_part[:],
                    counts_sb[:].rearrange("p (r l) -> p l r", r=RSL), axis=AX)

            # ---------------- Phase C: factors + entropies + mask -----------
            with (
                tc.tile_pool(name="pc_w", bufs=2) as pc_w,
                tc.tile_pool(name="pc_big", bufs=1) as pc_big,
                tc.tile_pool(name="pc_ps", bufs=2, space="PSUM") as pc_ps,
                tc.tile_pool(name="pc_io", bufs=4) as pc_io,
            ):
                covc = pc_big.tile([128, 1024], F32, name="covc")
                nc.sync.dma_start(covc[:], cov_out[:])
                # rs_own / centering
                rs_own = pc_w.tile([128, 1], F32, name="rs_own", bufs=1)
                tsel = pc_w.tile([128, 8], F32, name="tsel", bufs=1)
                nc.vector.tensor_mul(tsel[:], rowsum[:], sel_sb[:])
                nc.vector.reduce_sum(rs_own[:], tsel[:], axis=AX)
                nc.vector.tensor_single_scalar(rs_own[:], rs_own[:], -1.0 / float(D), A.mult)
                nc.vector.scalar_tensor_tensor(covc[:], rsb[:], rs_own[:], covc[:],
                                               A.mult, A.add)
                # diag mask: dmask[p, t*128+q] = sel[p, t] * (q == p)
                dmask = pc_big.tile([128, 1024], F32, name="dmask")
                for t in range(8):
                    nc.vector.tensor_single_scalar(
                        dmask[:, t * 128:(t + 1) * 128], id128[:], sel_sb[:, t:t + 1],
                        A.mult)
                dtmp = pc_w.tile([128, 1024], F32, name="dtmp")
                nc.vector.tensor_mul(dtmp[:], covc[:], dmask[:])
                d_own = pc_w.tile([128, 1], F32, name="d_own", bufs=1)
                nc.vector.reduce_sum(d_own[:], dtmp[:], axis=AX)
                nc.scalar.activation(d_own[:], d_own[:], AF.Sqrt)
                agm = pc_w.tile([128, 33], F32, name="agm", bufs=1)
                nc.vector.memset(agm[:], 0.0)
                nc.scalar.copy(agm[:, 0:1], d_own[:])
                nc.scalar.copy(agm[0:32, 1:33], bc_part[:])
                nc.sync.dma_start(agd_in[:], agm[:])
                nc.gpsimd.collective_compute(
                    "AllGather", A.bypass,
                    replica_groups=[list(range(N_CORES))],
                    ins=[agd_in.opt()], outs=[agd_out.opt()])
                nc.sync.dma_start(
                    dcol[:].rearrange("p (t o) -> p t o", o=1),
                    agd_out[:].rearrange("(t p) c -> p t c", p=128)[:, :, 0:1])
                rdc = pc_w.tile([128, 8], F32, name="rdc", bufs=1)
                nc.vector.reciprocal(rdc[:], dcol[:])
                _bcast_cols(nc, pc_w, pc_ps, rdc, rdb, id128)
                rd_own = pc_w.tile([128, 1], F32, name="rd_own", bufs=1)
                nc.vector.reciprocal(rd_own[:], d_own[:])
                t1 = pc_w.tile([128, 1024], F32, name="t1")
                nc.vector.tensor_mul(t1[:], covc[:], rdb[:])
                nc.vector.tensor_single_scalar(t1[:], t1[:], rd_own[:], A.mult)
                t1c = pc_w.tile([128, 1024], F32, name="t1c")
                nc.scalar.activation(t1c[:], t1[:], AF.Abs, accum_out=pvec[:, 3:4])
                nc.vector.tensor_single_scalar(pvec[:, 3:4], pvec[:, 3:4],
                                               1.0 / float(B), A.mult)

                # local entropies
                lnch = pc_big.tile([32, RSL * 32], F32, name="lnch")
                nc.scalar.activation(lnch[:], counts_sb[:], AF.Ln,
                                     scale=scal[0:32, 12:13], bias=epsb[0:32, :])
                nc.vector.tensor_mul(lnch[:], lnch[:], counts_sb[:])
                erp = pc_w.tile([32, RSL], F32, name="erp", bufs=1)
                nc.vector.reduce_sum(
                    erp[:], lnch[:].rearrange("p (r l) -> p r l", r=RSL), axis=AX)
                psS = pc_ps.tile([1, RSL], F32, name="psS")
                nc.tensor.matmul(psS[:], ones32[:], erp[:], start=True, stop=True)
                srow = pc_w.tile([1, RSL], F32, name="srow", bufs=1)
                nc.scalar.copy(srow[:], psS[:])
                psT = pc_ps.tile([128, 1], F32, name="psT")
                nc.tensor.transpose(psT[:], srow[:], id128[:1, :1])
                hloc = pc_w.tile([128, 1], F32, name="hloc", bufs=1)
                nc.scalar.copy(hloc[:], psT[:])
                nc.vector.tensor_mul(hloc[:], hloc[:], scal[:, 14:15])

                # batch entropy: sum gathered bc blocks (core r rows r*128..+32)
                agb = pc_w.tile([32, 8, 32], F32, name="agb", bufs=1)
                nc.sync.dma_start(
                    agb[:], agd_out[:].rearrange("(r p) c -> p r c", p=128)[0:32, :, 1:33])
                bcs = pc_w.tile([32, 32], F32, name="bcs", bufs=1)
                nc.vector.reduce_sum(
                    bcs[:], agb[:].rearrange("p r c -> p c r"), axis=AX)
                lnb = pc_w.tile([32, 32], F32, name="lnb", bufs=1)
                nc.scalar.activation(lnb[:], bcs[:], AF.Ln,
                                     scale=scal[0:32, 13:14], bias=epsb[0:32, :])
                nc.vector.tensor_mul(lnb[:], lnb[:], bcs[:])
                sb1 = pc_w.tile([32, 1], F32, name="sb1", bufs=1)
                nc.vector.reduce_sum(sb1[:], lnb[:], axis=AX)
                nc.gpsimd.partition_all_reduce(sb1[:], sb1[:], 32, _reduce_add())
                hbat = pc_w.tile([128, 1], F32, name="hbat", bufs=1)
                nc.gpsimd.partition_broadcast(hbat[:], sb1[0:1, :])
                nc.vector.tensor_mul(hbat[:], hbat[:], scal[:, 15:16])

                # f2' = max(f2, 1/f2); keep = f1/f2'; p = 1-keep; rkeep = 1/keep
                tA = pc_w.tile([128, 1], F32, name="tA", bufs=1)
                tB = pc_w.tile([128, 1], F32, name="tB", bufs=1)
                nc.vector.reciprocal(tA[:], hbat[:])
                f2 = pc_w.tile([128, 1], F32, name="f2", bufs=1)
                nc.vector.tensor_mul(f2[:], hloc[:], tA[:])
                nc.vector.reciprocal(tB[:], f2[:])
                nc.vector.tensor_max(f2[:], f2[:], tB[:])
                nc.vector.reciprocal(tB[:], f2[:])
                nc.vector.tensor_mul(pvec[:, 2:3], pvec[:, 3:4], tB[:])
                nc.vector.tensor_scalar(pvec[:, 0:1], pvec[:, 2:3], -1.0, 1.0,
                                        A.mult, A.add)
                nc.vector.reciprocal(pvec[:, 1:2], pvec[:, 2:3])

                if DEBUG:
                    dbg_sb = pc_w.tile([128, 16], F32, name="dbg_sb", bufs=1)
                    nc.scalar.copy(dbg_sb[:, 0:4], pvec[:])
                    nc.scalar.copy(dbg_sb[:, 4:5], hloc[:])
                    nc.scalar.copy(dbg_sb[:, 5:6], hbat[:])
                    nc.scalar.copy(dbg_sb[:, 6:14], scal[:, 4:12])
                    nc.scalar.copy(dbg_sb[:, 14:16], scal[:, 12:14])
                    nc.sync.dma_start(dbg.ap(), dbg_sb[:])
                    dbg2_sb = pc_w.tile([128, 48], F32, name="dbg2_sb", bufs=1)
                    nc.scalar.copy(dbg2_sb[:, 0:8], dcol[:])
                    nc.scalar.copy(dbg2_sb[:, 8:16], rdc[:])
                    nc.scalar.copy(dbg2_sb[:, 16:24], rsb[:, 0:8])
                    nc.scalar.copy(dbg2_sb[:, 24:32], rdb[:, 0:8])
                    nc.scalar.copy(dbg2_sb[:, 32:40], covc[:, 0:8])
                    nc.scalar.copy(dbg2_sb[:, 40:48], rowsum[:])
                    nc.sync.dma_start(dbg2.ap(), dbg2_sb[:])
                    dbg3_sb = pc_w.tile([128, 12], F32, name="dbg3_sb", bufs=1)
                    nc.scalar.copy(dbg3_sb[:, 0:4], t1[:, 0:4])
                    nc.scalar.copy(dbg3_sb[:, 4:8], covc[:, 0:4])
                    nc.scalar.copy(dbg3_sb[:, 8:12], rdc[:, 0:4])
                    nc.sync.dma_start(dbg3.ap(), dbg3_sb[:])

                # mask + scale
                CH = 2048
                for c in range((D // CH) if do_c else 0):
                    xm = pc_io.tile([128, CH], F32, name="xm")
                    um = pc_io.tile([128, CH], F32, name="um")
                    nc.sync.dma_start(xm[:], xr.ap()[:, c * CH:(c + 1) * CH])
                    nc.sync.dma_start(um[:], ur.ap()[:, c * CH:(c + 1) * CH])
                    nc.vector.tensor_single_scalar(um[:], um[:], pvec[:, 0:1], A.is_gt)
                    oc = pc_io.tile([128, CH], F32, name="oc")
                    nc.vector.scalar_tensor_tensor(oc[:], um[:], pvec[:, 1:2], xm[:],
                                                   A.mult, A.mult)
                    nc.sync.dma_start(out.ap()[:, c * CH:(c + 1) * CH], oc[:])

    nc.compile()
    return nc


def _reduce_add():
    from concourse import bass_isa
    return bass_isa.ReduceOp.add


def _reduce_max():
    from concourse import bass_isa
    return bass_isa.ReduceOp.max


def _bcast_cols(nc, sbuf_pool, psum_pool, vec8, dst, id128):
    """dst[p, t*128+q] = vec8[q, t]  (flatten [128,8] col-major, bcast to all
    partitions)."""
    import concourse.mybir as mybir
    F32 = mybir.dt.float32
    pt = psum_pool.tile([8, 128], F32, name="bc_pt")
    nc.tensor.transpose(pt[:8, :], vec8[:], id128[:])
    tr = sbuf_pool.tile([8, 128], F32, name="bc_tr", bufs=1)
    nc.scalar.copy(tr[:], pt[:8, :])
    flat = sbuf_pool.tile([1, 8 * 128], F32, name="bc_flat", bufs=1)
    for t in range(8):
        nc.sync.dma_start(flat[:, t * 128:(t + 1) * 128], tr[t:t + 1, :])
    nc.gpsimd.partition_broadcast(dst[:], flat[:])


def kernel(x, u):
    if "nc" not in _cache:
        _cache["nc"] = _build()
    nc = _cache["nc"]
    from concourse.bass_utils import run_bass_kernel_spmd

    x = np.asarray(x, dtype=np.float32)
    u = np.asarray(u, dtype=np.float32)
    orig_shape = x.shape
    xf = np.ascontiguousarray(x.reshape(B, D))
    uf = np.ascontiguousarray(u.reshape(B, D))
    in_maps = []
    for c in range(N_CORES):
        selv = np.zeros((128, 8), np.float32)
        selv[:, c] = 1.0
        in_maps.append({
            "xst": np.ascontiguousarray(xf[:, c * DSL:(c + 1) * DSL].T),
            "xr": np.ascontiguousarray(xf[c * RSL:(c + 1) * RSL, :]),
            "ur": np.ascontiguousarray(uf[c * RSL:(c + 1) * RSL, :]),
            "sel": selv,
        })
    res = run_bass_kernel_spmd(nc, in_maps, core_ids=list(range(N_CORES)))
    _cache["last_results"] = res
    outf = np.concatenate([res.results[c]["out"] for c in range(N_CORES)], axis=0)
    return outf.reshape(orig_shape)



# revision 21
# speedup vs baseline: 1.3336x; 1.3336x over previous
"""DifferentialDropout Trainium2 kernel (8-core SPMD), v3.

Reference semantics: per-row corrcoef factor f1, global-standardized 1000-bin
per-row histograms -> entropies -> per-row keep prob -> mask+scale.

Sharding:
  Phase A (D-shard): each core takes a 2048-col slice of temp [1024, 16384]
    shipped host-transposed and bf16-cast (xst [2048, 1024]) for the K-sharded
    cov GEMM (bf16 moving = 1 cyc/row on PE; corrcoef tolerates bf16 input
    rounding).  Own-row stats (rowsum/sumsq/min/max) from xr f32, chunked;
    AllGather [128,4] -> global mean/std/min/max -> SC/BC bin transform.
    cov partial -> ReduceScatter; centering as rank-1 correction post-RS.
    The d-column AllGather + f1 (mean |corr|) run DURING phase B.
  Phase B (B-shard): 128 own rows in 32 groups of 4.  idx prep in int16
    (trunc==floor, clamp skipped: only the global max lands at 1000 and
    1000>>5 is still 31).  Weights side: one-hot H planes over hi=idx>>5.
    Moving side: STEP planes U_j = [lo > j] (j=0..30) plus a ones column --
    steps are ONE op on every engine (DVE/Pool is_gt, Act saturated Sigmoid);
    exact counts recovered later by differencing adjacent columns (integers
    in fp32, exact).  Joint [4rx32, 4rx32] counts via 128 accumulating
    bf16 matmuls per group; diagonal blocks extracted to cnt_sb.
  Phase C: cnt diff -> counts, entropies via fold matmuls (W4/W32), batch
    count AllGather, keep prob, then mask+scale on Pool while DVE idles.
"""

import sys

sys.path.insert(0, "/opt/trn_rl_repo")

import numpy as np
import os

B = 1024
D = 16384
BINS = 1000
N_CORES = 8
DSL = D // N_CORES      # 2048
RSL = B // N_CORES      # 128
G = 4                   # rows per histogram group
N_G = RSL // G          # 32 groups
E = 128                 # e-positions per row per partition
F = G * E               # 512 free elems per group tile
LN2 = 0.6931471805599453

# plane split: DVE gets H[0, DH) one-hot + L-steps [0, DL); Act gets L-steps
# [DL, 31); Pool gets H[DH, 32) via 8-wide STTs.  31 step cols total (j=0..30).
DH = int(os.environ.get("K_DH", "24"))
DL = int(os.environ.get("K_DL", "16"))
MASK_POOL = int(os.environ.get("K_MPOOL", "0"))  # mask chunks on Pool (of 8)

_cache = {}


def _build():
    import concourse.mybir as mybir
    import concourse.tile as tile
    from concourse import bacc
    from concourse.masks import make_identity

    F32 = mybir.dt.float32
    BF16 = mybir.dt.bfloat16
    I16 = mybir.dt.int16
    A = mybir.AluOpType
    AF = mybir.ActivationFunctionType
    AX = mybir.AxisListType.X
    XY = mybir.AxisListType.XY

    nc = bacc.Bacc("TRN2", target_bir_lowering=False, debug=False,
                   num_devices=N_CORES)

    xst = nc.dram_tensor("xst", [DSL, B], F32, kind="ExternalInput")
    xr = nc.dram_tensor("xr", [RSL, D], F32, kind="ExternalInput")
    ur = nc.dram_tensor("ur", [RSL, D], F32, kind="ExternalInput")
    sel = nc.dram_tensor("sel", [128, 8], F32, kind="ExternalInput")
    out = nc.dram_tensor("out", [RSL, D], F32, kind="ExternalOutput")

    xr_v = xr.ap().rearrange("r (p e) -> p r e", p=128)   # [128, 128, 128]

    with tile.TileContext(nc) as tc:
        with (
            tc.tile_pool(name="const", bufs=1) as constp,
            tc.tile_pool(name="persist", bufs=1) as persist,
            tc.tile_pool(name="dram", bufs=1, space="DRAM") as dram,
        ):
            id128 = constp.tile([128, 128], F32, name="id128")
            make_identity(nc, id128[:])
            epsb = constp.tile([128, 1], F32, name="epsb")
            nc.vector.memset(epsb[:], 1e-30)
            # Act step bias: sigb[:, j] = -(64*j + 32); sigmoid(64*lo + sigb)
            sigb = constp.tile([128, 31], F32, name="sigb")
            for _j in range(31):
                nc.vector.memset(sigb[:, _j:_j + 1], -(64.0 * _j + 32.0))
            # q-pattern for Pool STT plane builds (int16): qpat[p, q, e] = q
            qpat = constp.tile([128, 8, E], I16, name="qpat")
            for _q in range(8):
                nc.vector.memset(qpat[:, _q, :], _q)
            # fold matrices: W4[32r+q, r'] = (r==r'), W32[32r+q, q'] = (q==q')
            W4 = constp.tile([128, 4], F32, name="W4")
            nc.vector.memset(W4[:], 0.0)
            for _r in range(4):
                nc.vector.memset(W4[32 * _r:32 * (_r + 1), _r:_r + 1], 1.0)
            W32 = constp.tile([128, 32], F32, name="W32")
            for _r in range(4):
                nc.scalar.copy(W32[32 * _r:32 * (_r + 1), :],
                               id128[32 * _r:32 * (_r + 1),
                                     32 * _r:32 * (_r + 1)])

            # persistent SBUF
            cnt_sb = persist.tile([128, N_G * 32], F32, name="cnt_sb")
            scal = persist.tile([128, 24], F32, name="scal")
            ag_sb = persist.tile([128, 4], F32, name="ag_sb")
            agg_sb = persist.tile([128, 8, 4], F32, name="agg_sb")
            rowsum = persist.tile([128, 8], F32, name="rowsum")
            rsb = persist.tile([128, 1024], F32, name="rsb")
            rdb = persist.tile([128, 1024], F32, name="rdb")
            dcol = persist.tile([128, 8], F32, name="dcol")
            sel_sb = persist.tile([128, 8], F32, name="sel_sb")
            pvec = persist.tile([128, 4], F32, name="pvec")  # p, rkeep, keep, f1
            bc_part = persist.tile([128, 32], F32, name="bc_part")
            hloc = persist.tile([128, 1], F32, name="hloc")

            nc.sync.dma_start(sel_sb[:], sel.ap())

            # DRAM bounces
            ag_in = dram.tile([128, 4], F32, name="ag_in")
            ag_out = dram.tile([1024, 4], F32, addr_space="Shared", name="ag_out")
            cov_in = dram.tile([1024, 1024], F32, name="cov_in")
            cov_out = dram.tile([128, 1024], F32, name="cov_out")
            agd_in = dram.tile([128, 1], F32, name="agd_in")
            agd_out = dram.tile([1024, 1], F32, addr_space="Shared", name="agd_out")
            agb_in = dram.tile([128, 32], F32, name="agb_in")
            agb_out = dram.tile([1024, 32], F32, addr_space="Shared", name="agb_out")
            hl_d = dram.tile([128, 1], F32, name="hl_d")

            # ---------------- Phase A: stats + GEMM --------------------------
            with (
                tc.tile_pool(name="pa_io", bufs=2) as pa_io,
                tc.tile_pool(name="pa_big", bufs=1) as pa_big,
                tc.tile_pool(name="pa_ps2", bufs=2, space="PSUM") as pa_ps2,
                tc.tile_pool(name="pa_w", bufs=2) as pa_w,
            ):
                # own-row stats from xr, chunked for DMA/compute overlap.
                # rowsum on Act (Identity accum over scratch), min/max on DVE.
                NSC = 4
                SCH = D // NSC
                rsa = pa_w.tile([128, NSC], F32, name="rsa", bufs=1)
                mna = pa_w.tile([128, NSC], F32, name="mna", bufs=1)
                mxa = pa_w.tile([128, NSC], F32, name="mxa", bufs=1)
                sqa = pa_w.tile([128, NSC], F32, name="sqa", bufs=1)
                for sc_ in range(NSC):
                    xrs = pa_io.tile([128, SCH], F32, name="xrs")
                    nc.sync.dma_start(xrs[:], xr.ap()[:, sc_ * SCH:(sc_ + 1) * SCH])
                    scr = pa_w.tile([128, SCH], F32, name="scr", tag="scr", bufs=2)
                    nc.scalar.activation(scr[:], xrs[:], AF.Identity,
                                         accum_out=rsa[:, sc_:sc_ + 1])
                    scr2 = pa_w.tile([128, SCH], F32, name="scr2", tag="scr", bufs=2)
                    nc.scalar.activation(scr2[:], xrs[:], AF.Square,
                                         accum_out=sqa[:, sc_:sc_ + 1])
                    nc.vector.tensor_reduce(mna[:, sc_:sc_ + 1], xrs[:], axis=AX,
                                            op=A.min, negate=True)
                    nc.vector.tensor_reduce(mxa[:, sc_:sc_ + 1], xrs[:], axis=AX,
                                            op=A.max)
                nc.vector.tensor_reduce(ag_sb[:, 0:1], rsa[:], axis=AX, op=A.add)
                nc.vector.tensor_reduce(ag_sb[:, 1:2], sqa[:], axis=AX, op=A.add)
                nc.vector.tensor_reduce(ag_sb[:, 2:3], mna[:], axis=AX, op=A.max)
                nc.vector.tensor_reduce(ag_sb[:, 3:4], mxa[:], axis=AX, op=A.max)
                nc.sync.dma_start(ag_in[:], ag_sb[:])
                nc.gpsimd.collective_compute(
                    "AllGather", A.bypass,
                    replica_groups=[list(range(N_CORES))],
                    ins=[ag_in.opt()], outs=[ag_out.opt()])
                nc.sync.dma_start(
                    agg_sb[:], ag_out[:].rearrange("(r p) c -> p r c", p=128))

                # GEMM on host-transposed slice; fp32r = exact fp32 at
                # 1 cyc/row for moving dim >= 256
                F32R = mybir.dt.float32r
                xst_sb = pa_big.tile([128, 16, 1024], F32R, name="xst_sb")
                for k in range(16):
                    nc.sync.dma_start(xst_sb[:, k, :],
                                      xst.ap()[k * 128:(k + 1) * 128, :].bitcast(F32R))
                covp = pa_big.tile([128, 8, 1024], F32, name="covp")
                for m in range(8):
                    for n2 in range(2):
                        pg = pa_ps2.tile([128, 512], F32, name="pg")
                        for k in range(16):
                            nc.tensor.matmul(
                                pg[:],
                                xst_sb[:, k, m * 128:(m + 1) * 128],
                                xst_sb[:, k, n2 * 512:(n2 + 1) * 512],
                                start=(k == 0), stop=(k == 15))
                        nc.scalar.copy(covp[:, m, n2 * 512:(n2 + 1) * 512], pg[:])
                nc.sync.dma_start(
                    cov_in[:].rearrange("(m p) j -> p m j", p=128), covp[:])
                nc.gpsimd.collective_compute(
                    "ReduceScatter", A.add,
                    replica_groups=[list(range(N_CORES))],
                    ins=[cov_in.opt()], outs=[cov_out.opt()])

                # -------- stats reduction + scalar constants --------
                nc.vector.tensor_copy(
                    rowsum[:].rearrange("p (r o) -> p r o", o=1),
                    agg_sb[:, :, 0:1])
                nc.vector.reduce_sum(scal[:, 16:17], rowsum[:], axis=AX)
                nc.gpsimd.partition_all_reduce(scal[:, 0:1], scal[:, 16:17], 128,
                                               _reduce_add())
                nc.vector.reduce_sum(
                    scal[:, 17:18], agg_sb[:, :, 1:2], axis=XY)
                nc.gpsimd.partition_all_reduce(scal[:, 1:2], scal[:, 17:18], 128,
                                               _reduce_add())
                nc.vector.reduce_sum(
                    scal[:, 18:19], agg_sb[:, :, 2:3], axis=XY, op=A.max)
                nc.gpsimd.partition_all_reduce(scal[:, 2:3], scal[:, 18:19], 128,
                                               _reduce_max())
                nc.vector.reduce_sum(
                    scal[:, 19:20], agg_sb[:, :, 3:4], axis=XY, op=A.max)
                nc.gpsimd.partition_all_reduce(scal[:, 3:4], scal[:, 19:20], 128,
                                               _reduce_max())

                N_f = float(B) * float(D)
                # mu = gsum/N
                nc.vector.tensor_single_scalar(scal[:, 4:5], scal[:, 0:1], 1.0 / N_f, A.mult)
                # var = (gss - gsum*mu)/(N-1); sd = sqrt
                nc.vector.tensor_mul(scal[:, 20:21], scal[:, 0:1], scal[:, 4:5])
                nc.vector.tensor_sub(scal[:, 20:21], scal[:, 1:2], scal[:, 20:21])
                nc.vector.tensor_single_scalar(scal[:, 20:21], scal[:, 20:21],
                                               1.0 / (N_f - 1.0), A.mult)
                nc.scalar.activation(scal[:, 5:6], scal[:, 20:21], AF.Sqrt)
                nc.vector.reciprocal(scal[:, 6:7], scal[:, 5:6])
                # lo = (tmin - mu)*rsd ; tmin = -negmn
                nc.vector.tensor_single_scalar(scal[:, 21:22], scal[:, 2:3], -1.0, A.mult)
                nc.vector.tensor_sub(scal[:, 21:22], scal[:, 21:22], scal[:, 4:5])
                nc.vector.tensor_mul(scal[:, 7:8], scal[:, 21:22], scal[:, 6:7])
                # hi = (tmax - mu)*rsd ; width = (hi - lo)/BINS
                nc.vector.tensor_sub(scal[:, 22:23], scal[:, 3:4], scal[:, 4:5])
                nc.vector.tensor_mul(scal[:, 22:23], scal[:, 22:23], scal[:, 6:7])
                nc.vector.tensor_sub(scal[:, 22:23], scal[:, 22:23], scal[:, 7:8])
                nc.vector.tensor_single_scalar(scal[:, 8:9], scal[:, 22:23],
                                               1.0 / BINS, A.mult)
                nc.vector.reciprocal(scal[:, 9:10], scal[:, 8:9])
                # SC = rsd*rwidth ; BC = -(mu*rsd + lo)*rwidth
                nc.vector.tensor_mul(scal[:, 10:11], scal[:, 6:7], scal[:, 9:10])
                nc.vector.tensor_mul(scal[:, 23:24], scal[:, 4:5], scal[:, 6:7])
                nc.vector.tensor_add(scal[:, 23:24], scal[:, 23:24], scal[:, 7:8])
                nc.vector.tensor_mul(scal[:, 23:24], scal[:, 23:24], scal[:, 9:10])
                # BC' = BC - 0.5: hw f32->int copies round-to-nearest, so
                # round(SC*x + BC - 0.5) == floor(SC*x + BC)
                nc.vector.tensor_scalar(scal[:, 11:12], scal[:, 23:24], -1.0, -0.5,
                                        A.mult, A.add)
                # entropy consts: rnw_l = 1/(width*D), rnw_b = 1/(width*N)
                nc.vector.tensor_single_scalar(scal[:, 16:17], scal[:, 8:9], float(D), A.mult)
                nc.vector.reciprocal(scal[:, 12:13], scal[:, 16:17])
                nc.vector.tensor_single_scalar(scal[:, 17:18], scal[:, 8:9], N_f, A.mult)
                nc.vector.reciprocal(scal[:, 13:14], scal[:, 17:18])
                nc.vector.tensor_single_scalar(scal[:, 14:15], scal[:, 12:13],
                                               -1.0 / LN2, A.mult)
                nc.vector.tensor_single_scalar(scal[:, 15:16], scal[:, 13:14],
                                               -1.0 / LN2, A.mult)
                # rsb = broadcast of flattened rowsum
                _bcast_cols(nc, pa_w, pa_ps2, rowsum, rsb, id128)

            # ---------------- Phase B: histogram + overlapped f1 -------------
            with (
                tc.tile_pool(name="hb_io", bufs=3) as hb_io,
                tc.tile_pool(name="hb_w", bufs=2) as hb_w,
                tc.tile_pool(name="hb_pl", bufs=2) as hb_pl,
                tc.tile_pool(name="hb_ps", bufs=4, space="PSUM") as hb_ps,
                tc.tile_pool(name="hb_c", bufs=1) as hb_c,
                tc.tile_pool(name="hb_cps", bufs=2, space="PSUM") as hb_cps,
            ):
                for g in range(N_G):
                    r0 = g * G
                    xch = hb_io.tile([128, G, E], F32, name="xch")
                    nc.sync.dma_start(xch[:], xr_v[:, r0:r0 + G, :])
                    xf = xch[:].rearrange("p a b -> p (a b)")

                    v = hb_w.tile([128, F], F32, name="v")
                    nc.scalar.activation(v[:], xf, AF.Identity,
                                         bias=scal[:, 11:12], scale=scal[:, 10:11])
                    vi = hb_w.tile([128, F], I16, name="vi", tag="pw", bufs=4)
                    nc.vector.tensor_copy(vi[:], v[:])
                    lo16 = hb_w.tile([128, F], I16, name="lo16", tag="pw", bufs=4)
                    nc.vector.tensor_single_scalar(lo16[:], vi[:], 31, A.bitwise_and)
                    # hi = floor((v+0.5)/32): i16 shifts fail the ISA check; via
                    # pow2 scale + rounding int copy ((v+.5)/32 - .5, all exact)
                    v32s = hb_w.tile([128, F], F32, name="v32s", tag="pw2", bufs=2)
                    nc.vector.tensor_scalar(v32s[:], v[:], 0.03125, -0.484375,
                                            A.mult, A.add)
                    hi16 = hb_w.tile([128, F], I16, name="hi16", tag="pw", bufs=4)
                    nc.vector.tensor_copy(hi16[:], v32s[:])
                    hi_v = hi16[:].rearrange("p (a b) -> p a b", a=G)
                    lo_v = lo16[:].rearrange("p (a b) -> p a b", a=G)

                    # planes PL[p, s, r, j, e]; s=0: H one-hot, s=1: col 0 ones,
                    # cols 1..31 = step [lo > j-1]
                    PL = hb_pl.tile([128, 2, G, 32, E], BF16, name="PL")
                    for q in range(DH):            # DVE H one-hot
                        nc.vector.tensor_single_scalar(
                            PL[:, 0, :, q, :], hi_v, float(q), A.is_equal)
                    for q in range(DH, 32):        # Pool H planes (plain TSP only)
                        nc.gpsimd.tensor_single_scalar(
                            PL[:, 0, :, q, :], hi_v, float(q), A.is_equal)
                    # ones col via is_gt(lo, -1) (always true; memset is 1x-slow)
                    nc.vector.tensor_single_scalar(
                        PL[:, 1, :, 0, :], lo_v, -1.0, A.is_gt)
                    for j in range(DL):            # DVE L steps
                        nc.vector.tensor_single_scalar(
                            PL[:, 1, :, j + 1, :], lo_v, float(j), A.is_gt)
                    for j in range(DL, 31):        # Act L steps (one op each)
                        nc.scalar.activation(PL[:, 1, :, j + 1, :], lo_v,
                                             AF.Sigmoid, bias=sigb[:, j:j + 1],
                                             scale=64.0)

                    # joint counts: accumulate outer products over e
                    ps = hb_ps.tile([128, 128], F32, name="ps")
                    for e in range(E):
                        nc.tensor.matmul(
                            ps[:],
                            PL[:, 0, :, :, e],
                            PL[:, 1, :, :, e],
                            start=(e == 0), stop=(e == E - 1))
                    for r in range(G):
                        if r % 2 == 0:
                            nc.vector.tensor_copy(
                                cnt_sb[32 * r:32 * (r + 1), g * 32:(g + 1) * 32],
                                ps[32 * r:32 * (r + 1), r * 32:(r + 1) * 32])
                        else:
                            nc.scalar.copy(
                                cnt_sb[32 * r:32 * (r + 1), g * 32:(g + 1) * 32],
                                ps[32 * r:32 * (r + 1), r * 32:(r + 1) * 32])

                    if g == 0:
                        # ---- overlapped: d-col AllGather + f1 (corr path) ----
                        covc = hb_c.tile([128, 1024], F32, name="covc")
                        nc.sync.dma_start(covc[:], cov_out[:])
                        rs_own = hb_c.tile([128, 1], F32, name="rs_own")
                        tsel = hb_c.tile([128, 8], F32, name="tsel")
                        nc.vector.tensor_mul(tsel[:], rowsum[:], sel_sb[:])
                        nc.vector.reduce_sum(rs_own[:], tsel[:], axis=AX)
                        nc.vector.tensor_single_scalar(rs_own[:], rs_own[:],
                                                       -1.0 / float(D), A.mult)
                        nc.vector.scalar_tensor_tensor(covc[:], rsb[:], rs_own[:],
                                                       covc[:], A.mult, A.add)
                        dmask = hb_c.tile([128, 1024], F32, name="dmask")
                        for t in range(8):
                            nc.vector.tensor_single_scalar(
                                dmask[:, t * 128:(t + 1) * 128], id128[:],
                                sel_sb[:, t:t + 1], A.mult)
                        dtmp = hb_c.tile([128, 1024], F32, name="dtmp")
                        nc.vector.tensor_mul(dtmp[:], covc[:], dmask[:])
                        d_own = hb_c.tile([128, 1], F32, name="d_own")
                        nc.vector.reduce_sum(d_own[:], dtmp[:], axis=AX)
                        nc.scalar.activation(d_own[:], d_own[:], AF.Sqrt)
                        nc.sync.dma_start(agd_in[:], d_own[:])
                        nc.gpsimd.collective_compute(
                            "AllGather", A.bypass,
                            replica_groups=[list(range(N_CORES))],
                            ins=[agd_in.opt()], outs=[agd_out.opt()])
                        nc.sync.dma_start(
                            dcol[:].rearrange("p (t o) -> p t o", o=1),
                            agd_out[:].rearrange("(t p) c -> p t c", p=128))
                        rdc = hb_c.tile([128, 8], F32, name="rdc")
                        nc.vector.reciprocal(rdc[:], dcol[:])
                        _bcast_cols(nc, hb_c, hb_cps, rdc, rdb, id128)
                        rd_own = hb_c.tile([128, 1], F32, name="rd_own")
                        nc.vector.reciprocal(rd_own[:], d_own[:])
                        t1 = hb_c.tile([128, 1024], F32, name="t1")
                        nc.vector.tensor_mul(t1[:], covc[:], rdb[:])
                        nc.vector.tensor_single_scalar(t1[:], t1[:], rd_own[:],
                                                       A.mult)
                        t1c = hb_c.tile([128, 1024], F32, name="t1c")
                        nc.scalar.activation(t1c[:], t1[:], AF.Abs,
                                             accum_out=pvec[:, 3:4])
                        nc.vector.tensor_single_scalar(pvec[:, 3:4], pvec[:, 3:4],
                                                       1.0 / float(B), A.mult)

            # ---------------- Phase C: entropies + keep + mask ---------------
            with (
                tc.tile_pool(name="pc_w", bufs=2) as pc_w,
                tc.tile_pool(name="pc_big", bufs=1) as pc_big,
                tc.tile_pool(name="pc_ps", bufs=2, space="PSUM") as pc_ps,
                tc.tile_pool(name="pc_io", bufs=4) as pc_io,
            ):
                # counts from step-diff: c_l = cnt[j=l] - cnt[j=l+1]; c_31 = cnt[31]
                cnt2 = pc_big.tile([128, N_G, 32], F32, name="cnt2")
                cv = cnt_sb[:].rearrange("p (g l) -> p g l", g=N_G)
                nc.vector.tensor_tensor(cnt2[:, :, 0:31], cv[:, :, 0:31],
                                        cv[:, :, 1:32], A.subtract)
                nc.vector.tensor_copy(cnt2[:, :, 31:32], cv[:, :, 31:32])
                cf = cnt2[:].rearrange("p g l -> p (g l)")
                # batch-count partial: fold over groups (cols), keep r-blocks
                nc.vector.reduce_sum(
                    bc_part[:],
                    cnt2[:].rearrange("p g l -> p l g"), axis=AX)
                nc.sync.dma_start(agb_in[:], bc_part[:])
                nc.gpsimd.collective_compute(
                    "AllGather", A.bypass,
                    replica_groups=[list(range(N_CORES))],
                    ins=[agb_in.opt()], outs=[agb_out.opt()])

                # local entropies from packed counts
                lnc = pc_big.tile([128, N_G * 32], F32, name="lnc")
                nc.scalar.activation(lnc[:], cf, AF.Ln,
                                     scale=scal[:, 12:13], bias=epsb[:])
                nc.vector.tensor_mul(lnc[:], lnc[:], cf)
                erp = pc_w.tile([128, N_G], F32, name="erp", bufs=1)
                nc.vector.reduce_sum(
                    erp[:], lnc[:].rearrange("p (g l) -> p g l", g=N_G), axis=AX)
                psE = pc_ps.tile([4, N_G], F32, name="psE")
                nc.tensor.matmul(psE[:], W4[:], erp[:], start=True, stop=True)
                srow = pc_w.tile([4, N_G], F32, name="srow", bufs=1)
                nc.scalar.copy(srow[:], psE[:])
                nc.sync.dma_start(
                    hl_d[:].rearrange("(g r) c -> r (g c)", r=4), srow[:])
                nc.sync.dma_start(hloc[:], hl_d[:])
                nc.vector.tensor_mul(hloc[:], hloc[:], scal[:, 14:15])

                # batch entropy: fold gathered bc over cores and r-blocks
                agb = pc_w.tile([128, 8, 32], F32, name="agb", bufs=1)
                nc.sync.dma_start(
                    agb[:], agb_out[:].rearrange("(r p) c -> p r c", p=128))
                bsum = pc_w.tile([128, 32], F32, name="bsum", bufs=1)
                nc.vector.reduce_sum(
                    bsum[:], agb[:].rearrange("p r c -> p c r"), axis=AX)
                psB = pc_ps.tile([32, 32], F32, name="psB")
                nc.tensor.matmul(psB[:], W32[:], bsum[:], start=True, stop=True)
                bcs = pc_w.tile([32, 32], F32, name="bcs", bufs=1)
                nc.scalar.copy(bcs[:], psB[:])
                lnb = pc_w.tile([32, 32], F32, name="lnb", bufs=1)
                nc.scalar.activation(lnb[:], bcs[:], AF.Ln,
                                     scale=scal[0:32, 13:14], bias=epsb[0:32, :])
                nc.vector.tensor_mul(lnb[:], lnb[:], bcs[:])
                sb1 = pc_w.tile([32, 1], F32, name="sb1", bufs=1)
                nc.vector.reduce_sum(sb1[:], lnb[:], axis=AX)
                nc.gpsimd.partition_all_reduce(sb1[:], sb1[:], 32, _reduce_add())
                hbat = pc_w.tile([128, 1], F32, name="hbat", bufs=1)
                nc.gpsimd.partition_broadcast(hbat[:], sb1[0:1, :])
                nc.vector.tensor_mul(hbat[:], hbat[:], scal[:, 15:16])

                # f2' = max(f2, 1/f2); keep = f1/f2'; p = 1-keep; rkeep = 1/keep
                tA = pc_w.tile([128, 1], F32, name="tA", bufs=1)
                tB = pc_w.tile([128, 1], F32, name="tB", bufs=1)
                nc.vector.reciprocal(tA[:], hbat[:])
                f2 = pc_w.tile([128, 1], F32, name="f2", bufs=1)
                nc.vector.tensor_mul(f2[:], hloc[:], tA[:])
                nc.vector.reciprocal(tB[:], f2[:])
                nc.vector.tensor_max(f2[:], f2[:], tB[:])
                nc.vector.reciprocal(tB[:], f2[:])
                nc.vector.tensor_mul(pvec[:, 2:3], pvec[:, 3:4], tB[:])
                nc.vector.tensor_scalar(pvec[:, 0:1], pvec[:, 2:3], -1.0, 1.0,
                                        A.mult, A.add)
                nc.vector.reciprocal(pvec[:, 1:2], pvec[:, 2:3])

                # mask + scale
                CH = 2048
                for c in range(D // CH):
                    xm = pc_io.tile([128, CH], F32, name="xm")
                    um = pc_io.tile([128, CH], F32, name="um")
                    nc.sync.dma_start(xm[:], xr.ap()[:, c * CH:(c + 1) * CH])
                    nc.sync.dma_start(um[:], ur.ap()[:, c * CH:(c + 1) * CH])
                    oc = pc_io.tile([128, CH], F32, name="oc")
                    if c < MASK_POOL:
                        nc.gpsimd.tensor_single_scalar(um[:], um[:], pvec[:, 0:1],
                                                       A.is_gt)
                        nc.gpsimd.scalar_tensor_tensor(oc[:], um[:], pvec[:, 1:2],
                                                       xm[:], A.mult, A.mult)
                    else:
                        nc.vector.tensor_single_scalar(um[:], um[:], pvec[:, 0:1],
                                                       A.is_gt)
                        nc.vector.scalar_tensor_tensor(oc[:], um[:], pvec[:, 1:2],
                                                       xm[:], A.mult, A.mult)
                    nc.sync.dma_start(out.ap()[:, c * CH:(c + 1) * CH], oc[:])

    nc.compile()
    return nc


def _reduce_add():
    from concourse import bass_isa
    return bass_isa.ReduceOp.add


def _reduce_max():
    from concourse import bass_isa
    return bass_isa.ReduceOp.max


def _bcast_cols(nc, sbuf_pool, psum_pool, vec8, dst, id128):
    """dst[p, t*128+q] = vec8[q, t]  (flatten [128,8] col-major, bcast to all
    partitions)."""
    import concourse.mybir as mybir
    F32 = mybir.dt.float32
    pt = psum_pool.tile([8, 128], F32, name="bc_pt")
    nc.tensor.transpose(pt[:8, :], vec8[:], id128[:])
    tr = sbuf_pool.tile([8, 128], F32, name="bc_tr", bufs=1)
    nc.scalar.copy(tr[:], pt[:8, :])
    flat = sbuf_pool.tile([1, 8 * 128], F32, name="bc_flat", bufs=1)
    for t in range(8):
        nc.sync.dma_start(flat[:, t * 128:(t + 1) * 128], tr[t:t + 1, :])
    nc.gpsimd.partition_broadcast(dst[:], flat[:])


def kernel(x, u):
    if "nc" not in _cache:
        _cache["nc"] = _build()
    nc = _cache["nc"]
    from concourse.bass_utils import run_bass_kernel_spmd
    import ml_dtypes

    x = np.asarray(x, dtype=np.float32)
    u = np.asarray(u, dtype=np.float32)
    orig_shape = x.shape
    xf = np.ascontiguousarray(x.reshape(B, D))
    uf = np.ascontiguousarray(u.reshape(B, D))
    in_maps = []
    for c in range(N_CORES):
        selv = np.zeros((128, 8), np.float32)
        selv[:, c] = 1.0
        in_maps.append({
            "xst": np.ascontiguousarray(xf[:, c * DSL:(c + 1) * DSL].T),
            "xr": np.ascontiguousarray(xf[c * RSL:(c + 1) * RSL, :]),
            "ur": np.ascontiguousarray(uf[c * RSL:(c + 1) * RSL, :]),
            "sel": selv,
        })
    res = run_bass_kernel_spmd(nc, in_maps, core_ids=list(range(N_CORES)))
    _cache["last_results"] = res
    outf = np.concatenate([res.results[c]["out"] for c in range(N_CORES)], axis=0)
    return outf.reshape(orig_shape)


# revision 34
# speedup vs baseline: 1.4617x; 1.0960x over previous
"""DifferentialDropout Trainium2 kernel (8-core SPMD), v3.

Reference semantics: per-row corrcoef factor f1, global-standardized 1000-bin
per-row histograms -> entropies -> per-row keep prob -> mask+scale.

Sharding:
  Phase A (D-shard): each core takes a 2048-col slice of temp [1024, 16384]
    shipped host-transposed and bf16-cast (xst [2048, 1024]) for the K-sharded
    cov GEMM (bf16 moving = 1 cyc/row on PE; corrcoef tolerates bf16 input
    rounding).  Own-row stats (rowsum/sumsq/min/max) from xr f32, chunked;
    AllGather [128,4] -> global mean/std/min/max -> SC/BC bin transform.
    cov partial -> ReduceScatter; centering as rank-1 correction post-RS.
    The d-column AllGather + f1 (mean |corr|) run DURING phase B.
  Phase B (B-shard): 128 own rows in 32 groups of 4.  idx prep in int16
    (trunc==floor, clamp skipped: only the global max lands at 1000 and
    1000>>5 is still 31).  Weights side: one-hot H planes over hi=idx>>5.
    Moving side: STEP planes U_j = [lo > j] (j=0..30) plus a ones column --
    steps are ONE op on every engine (DVE/Pool is_gt, Act saturated Sigmoid);
    exact counts recovered later by differencing adjacent columns (integers
    in fp32, exact).  Joint [4rx32, 4rx32] counts via 128 accumulating
    bf16 matmuls per group; diagonal blocks extracted to cnt_sb.
  Phase C: cnt diff -> counts, entropies via fold matmuls (W4/W32), batch
    count AllGather, keep prob, then mask+scale on Pool while DVE idles.
"""

import sys

sys.path.insert(0, "/opt/trn_rl_repo")

import numpy as np
import os

B = 1024
D = 16384
BINS = 1000
N_CORES = 8
DSL = D // N_CORES      # 2048
RSL = B // N_CORES      # 128
G = 4                   # rows per histogram group
N_G = RSL // G          # 32 groups
E = 128                 # e-positions per row per partition
F = G * E               # 512 free elems per group tile
LN2 = 0.6931471805599453

# plane split: DVE gets H[0, DH) one-hot + L-steps [0, DL); Act gets L-steps
# [DL, 31); Pool gets H[DH, 32) via 8-wide STTs.  31 step cols total (j=0..30).
DH = int(os.environ.get("K_DH", "22"))
DL = int(os.environ.get("K_DL", "20"))
MASK_POOL = int(os.environ.get("K_MPOOL", "0"))  # mask chunks on Pool (of 8)

_cache = {}


def _build():
    import concourse.mybir as mybir
    import concourse.tile as tile
    from concourse import bacc
    from concourse.masks import make_identity

    F32 = mybir.dt.float32
    BF16 = mybir.dt.bfloat16
    I16 = mybir.dt.int16
    A = mybir.AluOpType
    AF = mybir.ActivationFunctionType
    AX = mybir.AxisListType.X
    XY = mybir.AxisListType.XY

    nc = bacc.Bacc("TRN2", target_bir_lowering=False, debug=False,
                   num_devices=N_CORES)

    xst = nc.dram_tensor("xst", [DSL, B], F32, kind="ExternalInput")
    xr = nc.dram_tensor("xr", [RSL, D], F32, kind="ExternalInput")
    ur = nc.dram_tensor("ur", [RSL, D], F32, kind="ExternalInput")
    sel = nc.dram_tensor("sel", [128, 8], F32, kind="ExternalInput")
    out = nc.dram_tensor("out", [RSL, D], F32, kind="ExternalOutput")

    xr_v = xr.ap().rearrange("r (p e) -> p r e", p=128)   # [128, 128, 128]

    with tile.TileContext(nc) as tc:
        with (
            tc.tile_pool(name="const", bufs=1) as constp,
            tc.tile_pool(name="persist", bufs=1) as persist,
            tc.tile_pool(name="dram", bufs=1, space="DRAM") as dram,
        ):
            id128 = constp.tile([128, 128], F32, name="id128")
            make_identity(nc, id128[:])
            epsb = constp.tile([128, 1], F32, name="epsb")
            nc.vector.memset(epsb[:], 1e-30)
            one1 = constp.tile([128, 1], F32, name="one1")
            nc.vector.memset(one1[:], 1.0)
            # Act step bias: sigb[:, j] = -(64*j + 32); sigmoid(64*lo + sigb)
            sigb = constp.tile([128, 31], F32, name="sigb")
            for _j in range(31):
                nc.vector.memset(sigb[:, _j:_j + 1], -(64.0 * _j + 32.0))
            # q-pattern for Pool STT plane builds (int16): qpat[p, q, e] = q
            qpat = constp.tile([128, 8, E], I16, name="qpat")
            for _q in range(8):
                nc.vector.memset(qpat[:, _q, :], _q)
            # fold matrices: W4[32r+q, r'] = (r==r'), W32[32r+q, q'] = (q==q')
            W4 = constp.tile([128, 4], F32, name="W4")
            nc.vector.memset(W4[:], 0.0)
            for _r in range(4):
                nc.vector.memset(W4[32 * _r:32 * (_r + 1), _r:_r + 1], 1.0)
            W32 = constp.tile([128, 32], F32, name="W32")
            for _r in range(4):
                nc.scalar.copy(W32[32 * _r:32 * (_r + 1), :],
                               id128[32 * _r:32 * (_r + 1),
                                     32 * _r:32 * (_r + 1)])

            # persistent SBUF
            cnt_sb = persist.tile([128, N_G * 32], F32, name="cnt_sb")
            scal = persist.tile([128, 26], F32, name="scal")
            ag_sb = persist.tile([128, 4], F32, name="ag_sb")
            agg_sb = persist.tile([128, 8, 4], F32, name="agg_sb")
            rowsum = persist.tile([128, 8], F32, name="rowsum")
            rsb = persist.tile([128, 1024], F32, name="rsb")
            rdb = persist.tile([128, 1024], F32, name="rdb")
            dcol = persist.tile([128, 8], F32, name="dcol")
            sel_sb = persist.tile([128, 8], F32, name="sel_sb")
            pvec = persist.tile([128, 4], F32, name="pvec")  # p, rkeep, keep, f1
            bc_part = persist.tile([128, 32], F32, name="bc_part")
            hloc = persist.tile([128, 1], F32, name="hloc")

            nc.sync.dma_start(sel_sb[:], sel.ap())
            # mask chunk-0 prefetch (loaded during late phase B)
            pfx = persist.tile([128, 2048], F32, name="pfx")
            pfu = persist.tile([128, 2048], F32, name="pfu")

            # DRAM bounces
            ag_in = dram.tile([128, 4], F32, name="ag_in")
            ag_out = dram.tile([1024, 4], F32, addr_space="Shared", name="ag_out")
            cov_in = dram.tile([1024, 1024], F32, name="cov_in")
            cov_out = dram.tile([128, 1024], F32, name="cov_out")
            agd_in = dram.tile([128, 1], F32, name="agd_in")
            agd_out = dram.tile([1024, 1], F32, addr_space="Shared", name="agd_out")
            agb_in = dram.tile([128, 32], F32, name="agb_in")
            agb_out = dram.tile([1024, 32], F32, addr_space="Shared", name="agb_out")
            hl_d = dram.tile([128, 1], F32, name="hl_d")

            # ---------------- Phase A: stats + GEMM --------------------------
            with (
                tc.tile_pool(name="pa_io", bufs=2) as pa_io,
                tc.tile_pool(name="pa_big", bufs=1) as pa_big,
                tc.tile_pool(name="pa_ps2", bufs=2, space="PSUM") as pa_ps2,
                tc.tile_pool(name="pa_w", bufs=2) as pa_w,
            ):
                # own-row stats from xr, chunked for DMA/compute overlap.
                # rowsum on Act (Identity accum over scratch), min/max on DVE.
                NSC = 4
                SCH = D // NSC
                rsa = pa_w.tile([128, NSC], F32, name="rsa", bufs=1)
                mna = pa_w.tile([128, NSC], F32, name="mna", bufs=1)
                mxa = pa_w.tile([128, NSC], F32, name="mxa", bufs=1)
                sqa = pa_w.tile([128, NSC], F32, name="sqa", bufs=1)
                for sc_ in range(NSC):
                    xrs = pa_io.tile([128, SCH], F32, name="xrs")
                    nc.sync.dma_start(xrs[:], xr.ap()[:, sc_ * SCH:(sc_ + 1) * SCH])
                    scr = pa_w.tile([128, SCH], F32, name="scr", tag="scr", bufs=2)
                    nc.scalar.activation(scr[:], xrs[:], AF.Identity,
                                         accum_out=rsa[:, sc_:sc_ + 1])
                    scr2 = pa_w.tile([128, SCH], F32, name="scr2", tag="scr", bufs=2)
                    nc.scalar.activation(scr2[:], xrs[:], AF.Square,
                                         accum_out=sqa[:, sc_:sc_ + 1])
                    nc.vector.tensor_reduce(mna[:, sc_:sc_ + 1], xrs[:], axis=AX,
                                            op=A.min, negate=True)
                    nc.vector.tensor_reduce(mxa[:, sc_:sc_ + 1], xrs[:], axis=AX,
                                            op=A.max)
                nc.vector.tensor_reduce(ag_sb[:, 0:1], rsa[:], axis=AX, op=A.add)
                nc.vector.tensor_reduce(ag_sb[:, 1:2], sqa[:], axis=AX, op=A.add)
                nc.vector.tensor_reduce(ag_sb[:, 2:3], mna[:], axis=AX, op=A.max)
                nc.vector.tensor_reduce(ag_sb[:, 3:4], mxa[:], axis=AX, op=A.max)
                nc.sync.dma_start(ag_in[:], ag_sb[:])
                nc.gpsimd.collective_compute(
                    "AllGather", A.bypass,
                    replica_groups=[list(range(N_CORES))],
                    ins=[ag_in.opt()], outs=[ag_out.opt()])
                nc.sync.dma_start(
                    agg_sb[:], ag_out[:].rearrange("(r p) c -> p r c", p=128))

                # GEMM on host-transposed slice; fp32r = exact-enough fp32 at
                # 1 cyc/row for moving dim >= 256.  Loaded after the stats AG
                # so its 8MB of DMA doesn't delay ag_in on the queue.
                F32R = mybir.dt.float32r
                xst_sb = pa_big.tile([128, 16, 1024], F32R, name="xst_sb")
                for k in range(16):
                    nc.sync.dma_start(xst_sb[:, k, :],
                                      xst.ap()[k * 128:(k + 1) * 128, :].bitcast(F32R))
                covp = pa_big.tile([128, 8, 1024], F32, name="covp")
                for m in range(8):
                    for n2 in range(2):
                        pg = pa_ps2.tile([128, 512], F32, name="pg")
                        for k in range(16):
                            nc.tensor.matmul(
                                pg[:],
                                xst_sb[:, k, m * 128:(m + 1) * 128],
                                xst_sb[:, k, n2 * 512:(n2 + 1) * 512],
                                start=(k == 0), stop=(k == 15))
                        nc.scalar.copy(covp[:, m, n2 * 512:(n2 + 1) * 512], pg[:])
                nc.sync.dma_start(
                    cov_in[:].rearrange("(m p) j -> p m j", p=128), covp[:])
                nc.gpsimd.collective_compute(
                    "ReduceScatter", A.add,
                    replica_groups=[list(range(N_CORES))],
                    ins=[cov_in.opt()], outs=[cov_out.opt()])

                # -------- stats reduction + scalar constants --------
                nc.vector.tensor_copy(
                    rowsum[:].rearrange("p (r o) -> p r o", o=1),
                    agg_sb[:, :, 0:1])
                nc.vector.reduce_sum(scal[:, 16:17], rowsum[:], axis=AX)
                nc.gpsimd.partition_all_reduce(scal[:, 0:1], scal[:, 16:17], 128,
                                               _reduce_add())
                nc.vector.reduce_sum(
                    scal[:, 17:18], agg_sb[:, :, 1:2], axis=XY)
                nc.gpsimd.partition_all_reduce(scal[:, 1:2], scal[:, 17:18], 128,
                                               _reduce_add())
                nc.vector.reduce_sum(
                    scal[:, 18:19], agg_sb[:, :, 2:3], axis=XY, op=A.max)
                nc.gpsimd.partition_all_reduce(scal[:, 2:3], scal[:, 18:19], 128,
                                               _reduce_max())
                nc.vector.reduce_sum(
                    scal[:, 19:20], agg_sb[:, :, 3:4], axis=XY, op=A.max)
                nc.gpsimd.partition_all_reduce(scal[:, 3:4], scal[:, 19:20], 128,
                                               _reduce_max())

                N_f = float(B) * float(D)
                # mu = gsum/N
                nc.vector.tensor_single_scalar(scal[:, 4:5], scal[:, 0:1], 1.0 / N_f, A.mult)
                # var = (gss - gsum*mu)/(N-1); sd = sqrt
                nc.vector.tensor_mul(scal[:, 20:21], scal[:, 0:1], scal[:, 4:5])
                nc.vector.tensor_sub(scal[:, 20:21], scal[:, 1:2], scal[:, 20:21])
                nc.vector.tensor_single_scalar(scal[:, 20:21], scal[:, 20:21],
                                               1.0 / (N_f - 1.0), A.mult)
                nc.scalar.activation(scal[:, 5:6], scal[:, 20:21], AF.Sqrt)
                nc.vector.reciprocal(scal[:, 6:7], scal[:, 5:6])
                # lo = (tmin - mu)*rsd ; tmin = -negmn
                nc.vector.tensor_single_scalar(scal[:, 21:22], scal[:, 2:3], -1.0, A.mult)
                nc.vector.tensor_sub(scal[:, 21:22], scal[:, 21:22], scal[:, 4:5])
                nc.vector.tensor_mul(scal[:, 7:8], scal[:, 21:22], scal[:, 6:7])
                # hi = (tmax - mu)*rsd ; width = (hi - lo)/BINS
                nc.vector.tensor_sub(scal[:, 22:23], scal[:, 3:4], scal[:, 4:5])
                nc.vector.tensor_mul(scal[:, 22:23], scal[:, 22:23], scal[:, 6:7])
                nc.vector.tensor_sub(scal[:, 22:23], scal[:, 22:23], scal[:, 7:8])
                nc.vector.tensor_single_scalar(scal[:, 8:9], scal[:, 22:23],
                                               1.0 / BINS, A.mult)
                nc.vector.reciprocal(scal[:, 9:10], scal[:, 8:9])
                # SC = rsd*rwidth ; BC = -(mu*rsd + lo)*rwidth
                nc.vector.tensor_mul(scal[:, 10:11], scal[:, 6:7], scal[:, 9:10])
                nc.vector.tensor_mul(scal[:, 23:24], scal[:, 4:5], scal[:, 6:7])
                nc.vector.tensor_add(scal[:, 23:24], scal[:, 23:24], scal[:, 7:8])
                nc.vector.tensor_mul(scal[:, 23:24], scal[:, 23:24], scal[:, 9:10])
                # BC' = BC - 0.5: hw f32->int copies round-to-nearest, so
                # round(SC*x + BC - 0.5) == floor(SC*x + BC)
                nc.vector.tensor_scalar(scal[:, 11:12], scal[:, 23:24], -1.0, -0.5,
                                        A.mult, A.add)
                # entropy consts: rnw_l = 1/(width*D), rnw_b = 1/(width*N)
                nc.vector.tensor_single_scalar(scal[:, 16:17], scal[:, 8:9], float(D), A.mult)
                nc.vector.reciprocal(scal[:, 12:13], scal[:, 16:17])
                nc.vector.tensor_single_scalar(scal[:, 17:18], scal[:, 8:9], N_f, A.mult)
                nc.vector.reciprocal(scal[:, 13:14], scal[:, 17:18])
                nc.vector.tensor_single_scalar(scal[:, 14:15], scal[:, 12:13],
                                               -1.0 / LN2, A.mult)
                nc.vector.tensor_single_scalar(scal[:, 15:16], scal[:, 13:14],
                                               -1.0 / LN2, A.mult)
                # hi-extraction affine: SC/32 and (BC'+.5)/32 - .5 (pow2 exact)
                nc.vector.tensor_single_scalar(scal[:, 24:25], scal[:, 10:11],
                                               0.03125, A.mult)
                nc.vector.tensor_scalar(scal[:, 25:26], scal[:, 11:12], 0.03125,
                                        -0.484375, A.mult, A.add)
                # rsb = broadcast of flattened rowsum
                _bcast_cols(nc, pa_w, pa_ps2, rowsum, rsb, id128)

            # ---------------- Phase B: histogram + overlapped f1 -------------
            with (
                tc.tile_pool(name="hb_io", bufs=3) as hb_io,
                tc.tile_pool(name="hb_w", bufs=2) as hb_w,
                tc.tile_pool(name="hb_pl", bufs=2) as hb_pl,
                tc.tile_pool(name="hb_ps", bufs=4, space="PSUM") as hb_ps,
                tc.tile_pool(name="hb_c", bufs=1) as hb_c,
                tc.tile_pool(name="hb_cps", bufs=2, space="PSUM") as hb_cps,
            ):
                for g in range(N_G):
                    r0 = g * G
                    xch = hb_io.tile([128, G, E], F32, name="xch")
                    nc.sync.dma_start(xch[:], xr_v[:, r0:r0 + G, :])
                    xf = xch[:].rearrange("p a b -> p (a b)")

                    # idx prep: Act int16 output rounds-to-nearest on hw; the
                    # -0.5 baked into BC' turns round into floor.  H compares
                    # use the masked idx (hi<<5 = vi & 992) to skip a shift.
                    vi = hb_w.tile([128, F], I16, name="vi", tag="pw", bufs=4)
                    nc.scalar.activation(vi[:], xf, AF.Identity,
                                         bias=scal[:, 11:12], scale=scal[:, 10:11])
                    him = hb_w.tile([128, F], I16, name="him", tag="pw", bufs=4)
                    nc.vector.tensor_single_scalar(him[:], vi[:], 992, A.bitwise_and)
                    lo16 = hb_w.tile([128, F], I16, name="lo16", tag="pw", bufs=4)
                    nc.vector.tensor_single_scalar(lo16[:], vi[:], 31, A.bitwise_and)
                    hi_v = him[:].rearrange("p (a b) -> p a b", a=G)
                    lo_v = lo16[:].rearrange("p (a b) -> p a b", a=G)

                    # planes PL[p, s, r, j, e]; s=0: H one-hot, s=1: col 0 ones,
                    # cols 1..31 = step [lo > j-1]
                    PL = hb_pl.tile([128, 2, G, 32, E], BF16, name="PL")
                    for q in range(DH):            # DVE H one-hot
                        nc.vector.tensor_single_scalar(
                            PL[:, 0, :, q, :], hi_v, float(32 * q), A.is_equal)
                    for q in range(DH, 32):        # Pool H planes (plain TSP only)
                        nc.gpsimd.tensor_single_scalar(
                            PL[:, 0, :, q, :], hi_v, float(32 * q), A.is_equal)
                    # ones col: Act Copy with scale=0, bias=1
                    nc.scalar.activation(PL[:, 1, :, 0, :], lo_v, AF.Copy,
                                         bias=1.0, scale=0.0)
                    for j in range(DL):            # DVE L steps
                        nc.vector.tensor_single_scalar(
                            PL[:, 1, :, j + 1, :], lo_v, float(j), A.is_gt)
                    for j in range(DL, 31):        # Act L steps (one op each)
                        nc.scalar.activation(PL[:, 1, :, j + 1, :], lo_v,
                                             AF.Sigmoid, bias=sigb[:, j:j + 1],
                                             scale=64.0)

                    # joint counts: accumulate outer products over e
                    if g == N_G - 4:
                        nc.sync.dma_start(pfx[:], xr.ap()[:, 0:2048])
                        nc.sync.dma_start(pfu[:], ur.ap()[:, 0:2048])

                    ps = hb_ps.tile([128, 128], F32, name="ps")
                    for e in range(E):
                        nc.tensor.matmul(
                            ps[:],
                            PL[:, 0, :, :, e],
                            PL[:, 1, :, :, e],
                            start=(e == 0), stop=(e == E - 1))
                    for r in range(G):
                        nc.scalar.copy(
                            cnt_sb[32 * r:32 * (r + 1), g * 32:(g + 1) * 32],
                            ps[32 * r:32 * (r + 1), r * 32:(r + 1) * 32])

                    if g == 0:
                        # ---- overlapped: d-col AllGather + f1 (corr path) ----
                        covc = hb_c.tile([128, 1024], F32, name="covc")
                        nc.sync.dma_start(covc[:], cov_out[:])
                        rs_own = hb_c.tile([128, 1], F32, name="rs_own")
                        tsel = hb_c.tile([128, 8], F32, name="tsel")
                        nc.vector.tensor_mul(tsel[:], rowsum[:], sel_sb[:])
                        nc.vector.reduce_sum(rs_own[:], tsel[:], axis=AX)
                        nc.vector.tensor_single_scalar(rs_own[:], rs_own[:],
                                                       -1.0 / float(D), A.mult)
                        nc.vector.scalar_tensor_tensor(covc[:], rsb[:], rs_own[:],
                                                       covc[:], A.mult, A.add)
                        dmask = hb_c.tile([128, 1024], F32, name="dmask")
                        for t in range(8):
                            nc.vector.tensor_single_scalar(
                                dmask[:, t * 128:(t + 1) * 128], id128[:],
                                sel_sb[:, t:t + 1], A.mult)
                        dtmp = hb_c.tile([128, 1024], F32, name="dtmp")
                        nc.vector.tensor_mul(dtmp[:], covc[:], dmask[:])
                        d_own = hb_c.tile([128, 1], F32, name="d_own")
                        nc.vector.reduce_sum(d_own[:], dtmp[:], axis=AX)
                        nc.scalar.activation(d_own[:], d_own[:], AF.Sqrt)
                        nc.sync.dma_start(agd_in[:], d_own[:])
                        nc.gpsimd.collective_compute(
                            "AllGather", A.bypass,
                            replica_groups=[list(range(N_CORES))],
                            ins=[agd_in.opt()], outs=[agd_out.opt()])
                        nc.sync.dma_start(
                            dcol[:].rearrange("p (t o) -> p t o", o=1),
                            agd_out[:].rearrange("(t p) c -> p t c", p=128))
                        rdc = hb_c.tile([128, 8], F32, name="rdc")
                        nc.vector.reciprocal(rdc[:], dcol[:])
                        _bcast_cols(nc, hb_c, hb_cps, rdc, rdb, id128)
                        rd_own = hb_c.tile([128, 1], F32, name="rd_own")
                        nc.vector.reciprocal(rd_own[:], d_own[:])
                        t1 = hb_c.tile([128, 1024], F32, name="t1")
                        nc.vector.tensor_mul(t1[:], covc[:], rdb[:])
                        nc.vector.tensor_single_scalar(t1[:], t1[:], rd_own[:],
                                                       A.mult)
                        t1c = hb_c.tile([128, 1024], F32, name="t1c")
                        nc.scalar.activation(t1c[:], t1[:], AF.Abs,
                                             accum_out=pvec[:, 3:4])
                        nc.vector.tensor_single_scalar(pvec[:, 3:4], pvec[:, 3:4],
                                                       1.0 / float(B), A.mult)

            # ---------------- Phase C: entropies + keep + mask ---------------
            with (
                tc.tile_pool(name="pc_w", bufs=2) as pc_w,
                tc.tile_pool(name="pc_big", bufs=1) as pc_big,
                tc.tile_pool(name="pc_ps", bufs=2, space="PSUM") as pc_ps,
                tc.tile_pool(name="pc_io", bufs=4) as pc_io,
            ):
                # counts from step-diff: c_l = cnt[j=l] - cnt[j=l+1]; c_31 = cnt[31]
                cnt2 = pc_big.tile([128, N_G, 32], F32, name="cnt2")
                cv = cnt_sb[:].rearrange("p (g l) -> p g l", g=N_G)
                nc.vector.tensor_tensor(cnt2[:, :, 0:31], cv[:, :, 0:31],
                                        cv[:, :, 1:32], A.subtract)
                nc.vector.tensor_copy(cnt2[:, :, 31:32], cv[:, :, 31:32])
                cf = cnt2[:].rearrange("p g l -> p (g l)")
                # batch-count partial: fold over groups (cols), keep r-blocks
                nc.vector.reduce_sum(
                    bc_part[:],
                    cnt2[:].rearrange("p g l -> p l g"), axis=AX)
                nc.sync.dma_start(agb_in[:], bc_part[:])
                nc.gpsimd.collective_compute(
                    "AllGather", A.bypass,
                    replica_groups=[list(range(N_CORES))],
                    ins=[agb_in.opt()], outs=[agb_out.opt()])

                # local entropies from packed counts
                lnc = pc_big.tile([128, N_G * 32], F32, name="lnc")
                nc.scalar.activation(lnc[:], cf, AF.Ln,
                                     scale=scal[:, 12:13], bias=epsb[:])
                nc.vector.tensor_mul(lnc[:], lnc[:], cf)
                erp = pc_w.tile([128, N_G], F32, name="erp", bufs=1)
                nc.vector.reduce_sum(
                    erp[:], lnc[:].rearrange("p (g l) -> p g l", g=N_G), axis=AX)
                psE = pc_ps.tile([4, N_G], F32, name="psE")
                nc.tensor.matmul(psE[:], W4[:], erp[:], start=True, stop=True)
                srow = pc_w.tile([4, N_G], F32, name="srow", bufs=1)
                nc.scalar.copy(srow[:], psE[:])
                nc.sync.dma_start(
                    hl_d[:].rearrange("(g r) c -> r (g c)", r=4), srow[:])
                nc.sync.dma_start(hloc[:], hl_d[:])
                nc.vector.tensor_mul(hloc[:], hloc[:], scal[:, 14:15])

                # batch entropy: fold gathered bc over cores and r-blocks
                agb = pc_w.tile([128, 8, 32], F32, name="agb", bufs=1)
                nc.sync.dma_start(
                    agb[:], agb_out[:].rearrange("(r p) c -> p r c", p=128))
                bsum = pc_w.tile([128, 32], F32, name="bsum", bufs=1)
                nc.vector.reduce_sum(
                    bsum[:], agb[:].rearrange("p r c -> p c r"), axis=AX)
                psB = pc_ps.tile([32, 32], F32, name="psB")
                nc.tensor.matmul(psB[:], W32[:], bsum[:], start=True, stop=True)
                bcs = pc_w.tile([32, 32], F32, name="bcs", bufs=1)
                nc.scalar.copy(bcs[:], psB[:])
                lnb = pc_w.tile([32, 32], F32, name="lnb", bufs=1)
                nc.scalar.activation(lnb[:], bcs[:], AF.Ln,
                                     scale=scal[0:32, 13:14], bias=epsb[0:32, :])
                nc.vector.tensor_mul(lnb[:], lnb[:], bcs[:])
                sb1 = pc_w.tile([32, 1], F32, name="sb1", bufs=1)
                nc.vector.reduce_sum(sb1[:], lnb[:], axis=AX)
                nc.gpsimd.partition_all_reduce(sb1[:], sb1[:], 32, _reduce_add())
                hbat = pc_w.tile([128, 1], F32, name="hbat", bufs=1)
                nc.gpsimd.partition_broadcast(hbat[:], sb1[0:1, :])
                nc.vector.tensor_mul(hbat[:], hbat[:], scal[:, 15:16])

                # f2' = max(f2, 1/f2); keep = f1/f2'; p = 1-keep; rkeep = 1/keep
                tA = pc_w.tile([128, 1], F32, name="tA", bufs=1)
                tB = pc_w.tile([128, 1], F32, name="tB", bufs=1)
                nc.vector.reciprocal(tA[:], hbat[:])
                f2 = pc_w.tile([128, 1], F32, name="f2", bufs=1)
                nc.vector.tensor_mul(f2[:], hloc[:], tA[:])
                nc.vector.reciprocal(tB[:], f2[:])
                nc.vector.tensor_max(f2[:], f2[:], tB[:])
                nc.vector.reciprocal(tB[:], f2[:])
                nc.vector.tensor_mul(pvec[:, 2:3], pvec[:, 3:4], tB[:])
                nc.vector.tensor_scalar(pvec[:, 0:1], pvec[:, 2:3], -1.0, 1.0,
                                        A.mult, A.add)
                nc.vector.reciprocal(pvec[:, 1:2], pvec[:, 2:3])

                # mask + scale
                CH = 2048
                for c in range(D // CH):
                    if c == 0:
                        xm, um = pfx, pfu
                    else:
                        xm = pc_io.tile([128, CH], F32, name="xm")
                        um = pc_io.tile([128, CH], F32, name="um")
                        nc.sync.dma_start(xm[:], xr.ap()[:, c * CH:(c + 1) * CH])
                        nc.sync.dma_start(um[:], ur.ap()[:, c * CH:(c + 1) * CH])
                    oc = pc_io.tile([128, CH], F32, name="oc")
                    if c < MASK_POOL:
                        nc.gpsimd.tensor_single_scalar(um[:], um[:], pvec[:, 0:1],
                                                       A.is_gt)
                        nc.gpsimd.scalar_tensor_tensor(oc[:], um[:], pvec[:, 1:2],
                                                       xm[:], A.mult, A.mult)
                    else:
                        nc.vector.tensor_single_scalar(um[:], um[:], pvec[:, 0:1],
                                                       A.is_gt)
                        nc.vector.scalar_tensor_tensor(oc[:], um[:], pvec[:, 1:2],
                                                       xm[:], A.mult, A.mult)
                    nc.sync.dma_start(out.ap()[:, c * CH:(c + 1) * CH], oc[:])

    nc.compile()
    return nc


def _reduce_add():
    from concourse import bass_isa
    return bass_isa.ReduceOp.add


def _reduce_max():
    from concourse import bass_isa
    return bass_isa.ReduceOp.max


def _bcast_cols(nc, sbuf_pool, psum_pool, vec8, dst, id128):
    """dst[p, t*128+q] = vec8[q, t]  (flatten [128,8] col-major, bcast to all
    partitions)."""
    import concourse.mybir as mybir
    F32 = mybir.dt.float32
    pt = psum_pool.tile([8, 128], F32, name="bc_pt")
    nc.tensor.transpose(pt[:8, :], vec8[:], id128[:])
    tr = sbuf_pool.tile([8, 128], F32, name="bc_tr", bufs=1)
    nc.scalar.copy(tr[:], pt[:8, :])
    flat = sbuf_pool.tile([1, 8 * 128], F32, name="bc_flat", bufs=1)
    for t in range(8):
        nc.sync.dma_start(flat[:, t * 128:(t + 1) * 128], tr[t:t + 1, :])
    nc.gpsimd.partition_broadcast(dst[:], flat[:])


def kernel(x, u):
    if "nc" not in _cache:
        _cache["nc"] = _build()
    nc = _cache["nc"]
    from concourse.bass_utils import run_bass_kernel_spmd
    import ml_dtypes

    x = np.asarray(x, dtype=np.float32)
    u = np.asarray(u, dtype=np.float32)
    orig_shape = x.shape
    xf = np.ascontiguousarray(x.reshape(B, D))
    uf = np.ascontiguousarray(u.reshape(B, D))
    in_maps = []
    for c in range(N_CORES):
        selv = np.zeros((128, 8), np.float32)
        selv[:, c] = 1.0
        in_maps.append({
            "xst": np.ascontiguousarray(xf[:, c * DSL:(c + 1) * DSL].T),
            "xr": np.ascontiguousarray(xf[c * RSL:(c + 1) * RSL, :]),
            "ur": np.ascontiguousarray(uf[c * RSL:(c + 1) * RSL, :]),
            "sel": selv,
        })
    res = run_bass_kernel_spmd(nc, in_maps, core_ids=list(range(N_CORES)))
    _cache["last_results"] = res
    outf = np.concatenate([res.results[c]["out"] for c in range(N_CORES)], axis=0)
    return outf.reshape(orig_shape)
